# revision 12
# baseline (speedup 1.0000x reference)
"""Bass/Trainium2 kernel for nn_ConvFeature (deformable multi-branch 1D conv).

Strategy (pure data-parallel over batch, 2 sequences per core, 8 cores):
  h = emb[tokens]                      -- indirect-DMA row gather (fp16)
  off = 2*tanh(h @ wo + bo)            -- small matmuls on transposed h
  Deformable sampling with linear interpolation is expressed as a banded
  "tent" matrix:  val[:,t] = sum_t relu(1 - |pos - t|) * h[t,:]
  which is computed as dense fp16 matmuls over a 128-row halo window.
  Grouped conv = per-tap [128,256] matmuls accumulating in PSUM.
  LayerNorm via bn_stats/bn_aggr; masked mean-pool + final projection are
  folded into a per-position weighted dot product and one final
  partition-reduce matmul.

Sequence is tiled into 9 subtiles of 114 positions; each subtile's sample
window (+-7) fits a single 128-partition halo tile.
"""
import sys
import numpy as np

for _p in ('/opt/trn_rl_repo',):
    if _p not in sys.path:
        sys.path.insert(0, _p)

import concourse.bass as bass
import concourse.tile as tile
import concourse.bacc as bacc
from concourse import mybir
from concourse import bass_utils
from concourse.masks import make_identity

F32 = mybir.dt.float32
F16 = mybir.dt.float16
I32 = mybir.dt.int32
ACTF = mybir.ActivationFunctionType
ALU = mybir.AluOpType

# problem constants
B, S, W, VOCAB, G = 16, 1024, 512, 256, 2
WG = W // G                      # 256
KS = (3, 5, 7, 9)
NBR = len(KS)
TAPS = [float(t - (K - 1) // 2) for K in KS for t in range(K)]
NT = len(TAPS)                   # 24
BR_OFF = [0, 3, 8, 15, 24]       # global tap index ranges per branch
EPS = 1e-5

N_CORES = 8
BPC = B // N_CORES               # 2 sequences per core
HALO = 7
SUB = 114                        # positions per subtile
NSUBT = 9                        # 9*114 = 1026 >= 1024 (2 virtual tail positions)
SPAD = NSUBT * SUB               # 1026
ROWS = HALO + S + (SPAD - S) + HALO  # 1040 padded token rows
FREE = NT * SUB                  # 2736 tent columns
CC = W // 128                    # 4 channel chunks
SENT = 1 << 20                   # OOB sentinel token (gather skips, row stays 0)


def _build(use_bc: bool):
    nc = bacc.Bacc("TRN2", target_bir_lowering=False, debug=False,
                   num_devices=N_CORES)
    dt_in = lambda n, s, d: nc.dram_tensor(n, s, d, kind="ExternalInput").ap()
    tokp = dt_in("tokp", [BPC * ROWS, 1], I32)
    maskf = dt_in("maskf", [BPC * SPAD, 1], F32)
    emb16 = dt_in("emb16", [VOCAB, W], F16)
    woP = dt_in("woP", [128, CC * NT], F16)
    boP = dt_in("boP", [NT, 1], F32)
    wcP = dt_in("wcP", [128, NT * G * 2 * WG], F16)
    cmatP = dt_in("cmatP", [128, FREE], F32)
    gpwP = dt_in("gpwP", [NBR, W], F32)
    bcP = dt_in("bcP", [NBR, W], F32)
    foldP = dt_in("foldP", [1, NBR], F32)
    y = nc.dram_tensor("y", [BPC, NBR], F32, kind="ExternalOutput").ap()

    SL = [(i * 512, min(512, FREE - i * 512)) for i in range((FREE + 511) // 512)]

    with tile.TileContext(nc) as tc:
        with tc.tile_pool(name="const", bufs=1) as const, \
             tc.tile_pool(name="io", bufs=4) as io, \
             tc.tile_pool(name="mid", bufs=3) as mid, \
             tc.tile_pool(name="valp", bufs=2) as valp, \
             tc.tile_pool(name="lnp", bufs=3) as lnp, \
             tc.tile_pool(name="accp", bufs=2) as accp, \
             tc.tile_pool(name="psA", bufs=3, space="PSUM") as psA, \
             tc.tile_pool(name="psB", bufs=1, space="PSUM") as psB, \
             tc.tile_pool(name="psZ", bufs=1, space="PSUM") as psZ, \
             tc.tile_pool(name="psC", bufs=3, space="PSUM") as psC:

            # ---- constants ----
            ident = const.tile([128, 128], F16)
            make_identity(nc, ident[:])
            ones16 = const.tile([1, 128], F16)
            nc.vector.memset(ones16, 1.0)
            ones32 = const.tile([128, 1], F32)
            nc.vector.memset(ones32, 1.0)
            eps_sb = const.tile([128, 1], F32)
            nc.vector.memset(eps_sb, EPS)
            wo_sb = const.tile([128, CC * NT], F16)
            nc.sync.dma_start(out=wo_sb, in_=woP)
            bo_sb = const.tile([NT, 1], F32)
            nc.sync.dma_start(out=bo_sb, in_=boP)
            wc_sb = const.tile([128, NT * G * 2 * WG], F16)
            nc.sync.dma_start(out=wc_sb, in_=wcP)
            cmat_sb = const.tile([128, FREE], F32)
            nc.sync.dma_start(out=cmat_sb, in_=cmatP)
            fold_sb = const.tile([1, NBR], F32)
            nc.sync.dma_start(out=fold_sb, in_=foldP)
            gpw_rep = []
            for b in range(NBR):
                t = const.tile([128, W], F32, name=f"gpw{b}")
                nc.sync.dma_start(out=t, in_=gpwP[b:b + 1, :].to_broadcast((128, W)))
                gpw_rep.append(t)
            bc_rep = []
            if use_bc:
                for b in range(NBR):
                    t = const.tile([128, W], F32, name=f"bcr{b}")
                    nc.sync.dma_start(out=t, in_=bcP[b:b + 1, :].to_broadcast((128, W)))
                    bc_rep.append(t)

            for seq in range(BPC):
                acc = accp.tile([128, NBR + 1], F32, name=f"acc{seq}")
                nc.vector.memset(acc[:], 0.0)

                for j in range(NSUBT):
                    r0 = j * SUB          # padded row of halo start (= s0-7+HALO)
                    # ---- embedding gather for halo window ----
                    idx_sb = io.tile([128, 1], I32, tag="idx")
                    nc.sync.dma_start(out=idx_sb,
                                      in_=tokp[seq * ROWS + r0: seq * ROWS + r0 + 128, :])
                    h_halo = io.tile([128, W], F16, tag="hh")
                    nc.gpsimd.memset(h_halo[:], 0.0)
                    nc.gpsimd.indirect_dma_start(
                        out=h_halo[:], out_offset=None, in_=emb16[:],
                        in_offset=bass.IndirectOffsetOnAxis(ap=idx_sb[:, :1], axis=0),
                        bounds_check=VOCAB - 1, oob_is_err=False)
                    wcol = io.tile([SUB, 1], F32, tag="wcol")
                    nc.sync.dma_start(
                        out=wcol,
                        in_=maskf[seq * SPAD + j * SUB: seq * SPAD + (j + 1) * SUB, :])

                    # ---- transpose h (for offset matmul) via DMA xbar ----
                    hT = mid.tile([128, W], F16, tag="hT")
                    for cc in range(CC):
                        nc.scalar.dma_start(out=hT[:, cc * 128:(cc + 1) * 128],
                                            in_=h_halo[:, cc * 128:(cc + 1) * 128],
                                            transpose=True)

                    # ---- offsets: z[k, s] = sum_c wo[c,k] hT[c, s] ----
                    z_ps = psZ.tile([NT, SUB], F32, tag="z")
                    for cc in range(CC):
                        nc.tensor.matmul(z_ps[:],
                                         wo_sb[:, cc * NT:(cc + 1) * NT],
                                         hT[:, cc * 128 + HALO: cc * 128 + HALO + SUB],
                                         start=(cc == 0), stop=(cc == CC - 1))
                    off_t = mid.tile([NT, SUB], F16, tag="offt")
                    nc.scalar.activation(out=off_t[:], in_=z_ps[:], func=ACTF.Tanh,
                                         bias=bo_sb[:], scale=1.0)
                    off_row = mid.tile([1, FREE], F16, tag="offrow")
                    nc.sync.dma_start(out=off_row[:, :], in_=off_t[:, :])

                    # ---- tent = relu(1 - |cmat - 2*bcast(off)|) ----
                    d_sb = mid.tile([128, FREE], F16, tag="dsb")
                    bc_ps_l = {}
                    for (c0, n) in SL:
                        bc_ps = psB.tile([128, 512], F32, tag="bc", name=f"bc_{c0}")
                        nc.tensor.matmul(bc_ps[:, :n], ones16[:],
                                         off_row[:, c0:c0 + n], start=True, stop=True)
                        bc_ps_l[c0] = bc_ps
                    ad = mid.tile([128, FREE], F16, tag="ad")
                    tent = mid.tile([128, FREE], F16, tag="tent")
                    for (c0, n) in SL:
                        nc.vector.scalar_tensor_tensor(
                            out=d_sb[:, c0:c0 + n], in0=bc_ps_l[c0][:, :n],
                            scalar=-2.0, in1=cmat_sb[:, c0:c0 + n],
                            op0=ALU.mult, op1=ALU.add)
                        nc.scalar.activation(out=ad[:, c0:c0 + n],
                                             in_=d_sb[:, c0:c0 + n], func=ACTF.Abs)
                        nc.scalar.activation(out=tent[:, c0:c0 + n],
                                             in_=ad[:, c0:c0 + n], func=ACTF.Relu,
                                             bias=1.0, scale=-1.0)

                    # ---- gather: val[c, (k,s)] = sum_t h[t,c] tent[t,(k,s)] ----
                    val = [valp.tile([128, FREE], F16, tag=f"val{cc}", name=f"val{cc}_{seq}_{j}")
                           for cc in range(CC)]
                    di = 0
                    for cc in range(CC):
                        for (c0, n) in SL:
                            v_ps = psA.tile([128, 512], F32, tag="big")
                            nc.tensor.matmul(v_ps[:, :n],
                                             h_halo[:, cc * 128:(cc + 1) * 128],
                                             tent[:, c0:c0 + n], start=True, stop=True)
                            if di % 2:
                                nc.scalar.copy(out=val[cc][:, c0:c0 + n],
                                               in_=v_ps[:, :n])
                            else:
                                nc.vector.tensor_copy(out=val[cc][:, c0:c0 + n],
                                                      in_=v_ps[:, :n])
                            di += 1

                    # ---- per-branch conv + LN + pooled projection ----
                    for b in range(NBR):
                        psc = psC.tile([128, W], F32, tag="conv")
                        for g in range(G):
                            mms = [(kk, ic) for ic in range(2)
                                   for kk in range(BR_OFF[b], BR_OFF[b + 1])]
                            for mi, (kk, ic) in enumerate(mms):
                                blk = (kk * G + g) * 2 + ic
                                nc.tensor.matmul(
                                    psc[:SUB, g * WG:(g + 1) * WG],
                                    val[g * 2 + ic][:, kk * SUB:(kk + 1) * SUB],
                                    wc_sb[:, blk * WG:(blk + 1) * WG],
                                    start=(mi == 0), stop=(mi == len(mms) - 1))
                        if use_bc:
                            nc.vector.tensor_tensor(out=psc[:SUB, :], in0=psc[:SUB, :],
                                                    in1=bc_rep[b][:SUB, :], op=ALU.add)
                        st = lnp.tile([SUB, 6], F32, tag="st")
                        nc.vector.bn_stats(out=st[:], in_=psc[:SUB, :])
                        mv = lnp.tile([SUB, 2], F32, tag="mv")
                        nc.vector.bn_aggr(out=mv[:], in_=st[:])
                        sd = lnp.tile([SUB, 1], F32, tag="sd")
                        nc.scalar.activation(out=sd[:], in_=mv[:, 1:2], func=ACTF.Sqrt,
                                             bias=eps_sb[:SUB], scale=1.0)
                        rstd = lnp.tile([SUB, 1], F32, tag="rstd")
                        nc.vector.reciprocal(out=rstd[:], in_=sd[:])
                        nmr = lnp.tile([SUB, 1], F32, tag="nmr")
                        nc.vector.tensor_scalar(out=nmr[:], in0=mv[:, 0:1],
                                                scalar1=rstd[:], scalar2=-1.0,
                                                op0=ALU.mult, op1=ALU.mult)
                        xhat = lnp.tile([SUB, W], F32, tag="xhat")
                        nc.scalar.activation(out=xhat[:], in_=psc[:SUB, :],
                                             func=ACTF.Identity,
                                             bias=nmr[:], scale=rstd[:])
                        scr = lnp.tile([SUB, W], F32, tag="scr")
                        qt = lnp.tile([SUB, 1], F32, tag="qt")
                        nc.vector.scalar_tensor_tensor(
                            out=scr[:], in0=xhat[:], scalar=wcol[:],
                            in1=gpw_rep[b][:SUB, :], op0=ALU.mult, op1=ALU.mult,
                            accum_out=qt[:])
                        nc.vector.tensor_tensor(out=acc[:SUB, b:b + 1],
                                                in0=acc[:SUB, b:b + 1], in1=qt[:],
                                                op=ALU.add)
                    nc.vector.tensor_tensor(out=acc[:SUB, NBR:NBR + 1],
                                            in0=acc[:SUB, NBR:NBR + 1], in1=wcol[:],
                                            op=ALU.add)

                # ---- finalize sequence: partition-reduce, divide, project ----
                f_ps_t = psZ.tile([NT, SUB], F32, tag="z", name=f"fin{seq}")
                f_ps = f_ps_t[:NBR + 1, :1]
                nc.tensor.matmul(f_ps[:], acc[:], ones32[:], start=True, stop=True)
                f_sb = accp.tile([NBR + 1, 1], F32, tag="fsb")
                nc.vector.tensor_copy(out=f_sb[:], in_=f_ps[:])
                frow = accp.tile([1, NBR + 1], F32, tag="frow")
                nc.sync.dma_start(out=frow[:, :], in_=f_sb[:, :])
                fmax = accp.tile([1, 1], F32, tag="fmax")
                nc.vector.tensor_scalar_max(out=fmax[:], in0=frow[:, NBR:NBR + 1],
                                            scalar1=1.0)
                rec = accp.tile([1, 1], F32, tag="rec")
                nc.vector.reciprocal(out=rec[:], in_=fmax[:])
                feats = accp.tile([1, NBR], F32, tag="feats")
                nc.vector.tensor_scalar(out=feats[:], in0=frow[:, :NBR],
                                        scalar1=rec[:], scalar2=1.0,
                                        op0=ALU.mult, op1=ALU.mult)
                nc.vector.tensor_tensor(out=feats[:], in0=feats[:], in1=fold_sb[:],
                                        op=ALU.add)
                nc.sync.dma_start(out=y[seq:seq + 1, :], in_=feats[:])
    nc.compile()
    return nc


_CACHE = {}


def _get_nc(use_bc: bool):
    if use_bc not in _CACHE:
        _CACHE[use_bc] = _build(use_bc)
    return _CACHE[use_bc]


def kernel(tokens, mask, emb, branch_params):
    tokens = np.asarray(tokens)
    mask = np.asarray(mask)
    emb = np.asarray(emb, dtype=np.float32)
    bps = [{k: np.asarray(v, dtype=np.float32) for k, v in bp.items()}
           for bp in branch_params]

    # ---- host-side packing (shared across cores) ----
    emb16 = emb.astype(np.float16)
    wo_all = np.concatenate([bp['wo'] for bp in bps], axis=1)        # [512, 24]
    woP = np.concatenate([wo_all[cc * 128:(cc + 1) * 128, :] for cc in range(CC)],
                         axis=1).astype(np.float16)                  # [128, 96]
    boP = np.concatenate([bp['bo'] for bp in bps])[:, None].astype(np.float32)
    wcP = np.zeros((128, NT * G * 2 * WG), np.float16)
    for bi, bp in enumerate(bps):
        K = KS[bi]
        for kj in range(K):
            kk = BR_OFF[bi] + kj
            for g in range(G):
                for ic in range(2):
                    blk = (kk * G + g) * 2 + ic
                    # wc[g, o, i, k] -> lhsT [i(128), o(256)]
                    wcP[:, blk * WG:(blk + 1) * WG] = \
                        bp['wc'][g, :, ic * 128:(ic + 1) * 128, kj].T
    cmatP = np.zeros((128, FREE), np.float32)
    tl = np.arange(128, dtype=np.float32)[:, None]
    sl_ = np.arange(SUB, dtype=np.float32)[None, :]
    for kk in range(NT):
        cmatP[:, kk * SUB:(kk + 1) * SUB] = tl - HALO - sl_ - TAPS[kk]
    gpwP = np.stack([bp['ln_g'] * bp['pw'] for bp in bps]).astype(np.float32)
    bcP = np.stack([bp['bc'] for bp in bps]).astype(np.float32)
    foldP = np.array([[float((bp['ln_b'] * bp['pw']).sum() + bp['pb'])
                       for bp in bps]], np.float32)
    use_bc = bool(np.any(bcP))

    in_maps = []
    for c in range(N_CORES):
        tokp = np.full((BPC, ROWS), SENT, np.int32)
        tokp[:, HALO:HALO + S] = tokens[c * BPC:(c + 1) * BPC].astype(np.int32)
        mf = np.zeros((BPC, SPAD), np.float32)
        mf[:, :S] = 1.0 - mask[c * BPC:(c + 1) * BPC].astype(np.float32)
        in_maps.append(dict(
            tokp=tokp.reshape(-1, 1), maskf=mf.reshape(-1, 1), emb16=emb16,
            woP=woP, boP=boP, wcP=wcP, cmatP=cmatP, gpwP=gpwP, bcP=bcP,
            foldP=foldP))

    nc = _get_nc(use_bc)
    res = bass_utils.run_bass_kernel_spmd(nc, in_maps, core_ids=list(range(N_CORES)))
    out = np.concatenate([res.results[c]["y"] for c in range(N_CORES)], axis=0)
    return out.astype(np.float32)


# revision 14
# speedup vs baseline: 4619.0802x; 4619.0802x over previous
"""Bass/Trainium2 kernel for nn_ConvFeature (deformable multi-branch 1D conv).

Strategy (pure data-parallel over batch, 2 sequences per core, 8 cores):
  h = emb[tokens]                      -- indirect-DMA row gather (fp16)
  off = 2*tanh(h @ wo + bo)            -- small matmuls on transposed h
  Deformable sampling with linear interpolation is expressed as a banded
  "tent" matrix:  val[:,t] = sum_t relu(1 - |pos - t|) * h[t,:]
  which is computed as dense fp16 matmuls over a 128-row halo window.
  Grouped conv = per-tap [128,256] matmuls accumulating in PSUM.
  LayerNorm via bn_stats/bn_aggr; masked mean-pool + final projection are
  folded into a per-position weighted dot product and one final
  partition-reduce matmul.

Sequence is tiled into 9 subtiles of 114 positions; each subtile's sample
window (+-7) fits a single 128-partition halo tile.
"""
import sys
import numpy as np

for _p in ('/opt/trn_rl_repo',):
    if _p not in sys.path:
        sys.path.insert(0, _p)

import concourse.bass as bass
import concourse.tile as tile
import concourse.bacc as bacc
from concourse import mybir
from concourse import bass_utils
from concourse.masks import make_identity

F32 = mybir.dt.float32
F16 = mybir.dt.float16
I32 = mybir.dt.int32
ACTF = mybir.ActivationFunctionType
ALU = mybir.AluOpType

# problem constants
B, S, W, VOCAB, G = 16, 1024, 512, 256, 2
WG = W // G                      # 256
KS = (3, 5, 7, 9)
NBR = len(KS)
TAPS = [float(t - (K - 1) // 2) for K in KS for t in range(K)]
NT = len(TAPS)                   # 24
BR_OFF = [0, 3, 8, 15, 24]       # global tap index ranges per branch
EPS = 1e-5

N_CORES = 8
BPC = B // N_CORES               # 2 sequences per core
HALO = 7
SUB = 114                        # positions per subtile
NSUBT = 9                        # 9*114 = 1026 >= 1024 (2 virtual tail positions)
SPAD = NSUBT * SUB               # 1026
ROWS = HALO + S + (SPAD - S) + HALO  # 1040 padded token rows
FREE = NT * SUB                  # 2736 tent columns
CC = W // 128                    # 4 channel chunks
SENT = 1 << 20                   # OOB sentinel token (gather skips, row stays 0)


def _build(use_bc: bool):
    nc = bacc.Bacc("TRN2", target_bir_lowering=False, debug=False,
                   num_devices=N_CORES)
    dt_in = lambda n, s, d: nc.dram_tensor(n, s, d, kind="ExternalInput").ap()
    tokp = dt_in("tokp", [BPC * ROWS, 1], I32)
    maskf = dt_in("maskf", [BPC * SPAD, 1], F32)
    emb16 = dt_in("emb16", [VOCAB, W], F16)
    woP = dt_in("woP", [128, CC * NT], F16)
    boP = dt_in("boP", [NT, 1], F32)
    wcP = dt_in("wcP", [128, NT * G * 2 * WG], F16)
    cmatP = dt_in("cmatP", [128, FREE], F32)
    gpwP = dt_in("gpwP", [NBR, W], F32)
    bcP = dt_in("bcP", [NBR, W], F32)
    foldP = dt_in("foldP", [1, NBR], F32)
    y = nc.dram_tensor("y", [BPC, NBR], F32, kind="ExternalOutput").ap()

    SL = [(i * 512, min(512, FREE - i * 512)) for i in range((FREE + 511) // 512)]

    with tile.TileContext(nc) as tc:
        with tc.tile_pool(name="const", bufs=1) as const, \
             tc.tile_pool(name="io", bufs=4) as io, \
             tc.tile_pool(name="mid", bufs=3) as mid, \
             tc.tile_pool(name="valp", bufs=2) as valp, \
             tc.tile_pool(name="lnp", bufs=3) as lnp, \
             tc.tile_pool(name="accp", bufs=2) as accp, \
             tc.tile_pool(name="psA", bufs=3, space="PSUM") as psA, \
             tc.tile_pool(name="psB", bufs=1, space="PSUM") as psB, \
             tc.tile_pool(name="psZ", bufs=1, space="PSUM") as psZ, \
             tc.tile_pool(name="psC", bufs=3, space="PSUM") as psC:

            # ---- constants ----
            ident = const.tile([128, 128], F16)
            make_identity(nc, ident[:])
            ones16 = const.tile([1, 128], F16)
            nc.vector.memset(ones16, 1.0)
            ones32 = const.tile([128, 1], F32)
            nc.vector.memset(ones32, 1.0)
            eps_sb = const.tile([128, 1], F32)
            nc.vector.memset(eps_sb, EPS)
            wo_sb = const.tile([128, CC * NT], F16)
            nc.sync.dma_start(out=wo_sb, in_=woP)
            bo_sb = const.tile([NT, 1], F32)
            nc.sync.dma_start(out=bo_sb, in_=boP)
            wc_sb = const.tile([128, NT * G * 2 * WG], F16)
            nc.sync.dma_start(out=wc_sb, in_=wcP)
            cmat_sb = const.tile([128, FREE], F32)
            nc.sync.dma_start(out=cmat_sb, in_=cmatP)
            fold_sb = const.tile([1, NBR], F32)
            nc.sync.dma_start(out=fold_sb, in_=foldP)
            gpw_rep = []
            for b in range(NBR):
                t = const.tile([128, W], F32, name=f"gpw{b}")
                nc.sync.dma_start(out=t, in_=gpwP[b:b + 1, :].to_broadcast((128, W)))
                gpw_rep.append(t)
            bc_rep = []
            if use_bc:
                for b in range(NBR):
                    t = const.tile([128, W], F32, name=f"bcr{b}")
                    nc.sync.dma_start(out=t, in_=bcP[b:b + 1, :].to_broadcast((128, W)))
                    bc_rep.append(t)

            for seq in range(BPC):
                acc = accp.tile([128, NBR + 1], F32, name=f"acc{seq}")
                nc.vector.memset(acc[:], 0.0)

                for j in range(NSUBT):
                    r0 = j * SUB          # padded row of halo start (= s0-7+HALO)
                    # ---- embedding gather for halo window ----
                    idx_sb = io.tile([128, 1], I32, tag="idx")
                    nc.sync.dma_start(out=idx_sb,
                                      in_=tokp[seq * ROWS + r0: seq * ROWS + r0 + 128, :])
                    h_halo = io.tile([128, W], F16, tag="hh")
                    nc.gpsimd.memset(h_halo[:], 0.0)
                    nc.gpsimd.indirect_dma_start(
                        out=h_halo[:], out_offset=None, in_=emb16[:],
                        in_offset=bass.IndirectOffsetOnAxis(ap=idx_sb[:, :1], axis=0),
                        bounds_check=VOCAB - 1, oob_is_err=False)
                    wcol = io.tile([SUB, 1], F32, tag="wcol")
                    nc.sync.dma_start(
                        out=wcol,
                        in_=maskf[seq * SPAD + j * SUB: seq * SPAD + (j + 1) * SUB, :])

                    # ---- transpose h (for offset matmul) via DMA xbar ----
                    hT = mid.tile([128, W], F16, tag="hT")
                    for cc in range(CC):
                        nc.scalar.dma_start(out=hT[:, cc * 128:(cc + 1) * 128],
                                            in_=h_halo[:, cc * 128:(cc + 1) * 128],
                                            transpose=True)

                    # ---- offsets: z[k, s] = sum_c wo[c,k] hT[c, s] ----
                    z_ps = psZ.tile([NT, SUB], F32, tag="z")
                    for cc in range(CC):
                        nc.tensor.matmul(z_ps[:],
                                         wo_sb[:, cc * NT:(cc + 1) * NT],
                                         hT[:, cc * 128 + HALO: cc * 128 + HALO + SUB],
                                         start=(cc == 0), stop=(cc == CC - 1))
                    off_t = mid.tile([NT, SUB], F16, tag="offt")
                    nc.scalar.activation(out=off_t[:], in_=z_ps[:], func=ACTF.Tanh,
                                         bias=bo_sb[:], scale=1.0)
                    off_row = mid.tile([1, FREE], F16, tag="offrow")
                    nc.sync.dma_start(out=off_row[:, :], in_=off_t[:, :])

                    # ---- tent = relu(1 - |cmat - 2*bcast(off)|) ----
                    d_sb = mid.tile([128, FREE], F16, tag="dsb")
                    bc_ps_l = {}
                    for (c0, n) in SL:
                        bc_ps = psB.tile([128, 512], F32, tag="bc", name=f"bc_{c0}")
                        nc.tensor.matmul(bc_ps[:, :n], ones16[:],
                                         off_row[:, c0:c0 + n], start=True, stop=True)
                        bc_ps_l[c0] = bc_ps
                    ad = mid.tile([128, FREE], F16, tag="ad")
                    tent = mid.tile([128, FREE], F16, tag="tent")
                    for (c0, n) in SL:
                        nc.vector.scalar_tensor_tensor(
                            out=d_sb[:, c0:c0 + n], in0=bc_ps_l[c0][:, :n],
                            scalar=-2.0, in1=cmat_sb[:, c0:c0 + n],
                            op0=ALU.mult, op1=ALU.add)
                        nc.scalar.activation(out=ad[:, c0:c0 + n],
                                             in_=d_sb[:, c0:c0 + n], func=ACTF.Abs)
                        nc.scalar.activation(out=tent[:, c0:c0 + n],
                                             in_=ad[:, c0:c0 + n], func=ACTF.Relu,
                                             bias=1.0, scale=-1.0)

                    # ---- gather: val[c, (k,s)] = sum_t h[t,c] tent[t,(k,s)] ----
                    val = [valp.tile([128, FREE], F16, tag=f"val{cc}", name=f"val{cc}_{seq}_{j}")
                           for cc in range(CC)]
                    di = 0
                    for cc in range(CC):
                        for (c0, n) in SL:
                            v_ps = psA.tile([128, 512], F32, tag="big")
                            nc.tensor.matmul(v_ps[:, :n],
                                             h_halo[:, cc * 128:(cc + 1) * 128],
                                             tent[:, c0:c0 + n], start=True, stop=True)
                            if di % 2:
                                nc.scalar.copy(out=val[cc][:, c0:c0 + n],
                                               in_=v_ps[:, :n])
                            else:
                                nc.vector.tensor_copy(out=val[cc][:, c0:c0 + n],
                                                      in_=v_ps[:, :n])
                            di += 1

                    # ---- per-branch conv + LN + pooled projection ----
                    for b in range(NBR):
                        psc = psC.tile([128, W], F32, tag="conv")
                        for g in range(G):
                            mms = [(kk, ic) for ic in range(2)
                                   for kk in range(BR_OFF[b], BR_OFF[b + 1])]
                            for mi, (kk, ic) in enumerate(mms):
                                blk = (kk * G + g) * 2 + ic
                                nc.tensor.matmul(
                                    psc[:SUB, g * WG:(g + 1) * WG],
                                    val[g * 2 + ic][:, kk * SUB:(kk + 1) * SUB],
                                    wc_sb[:, blk * WG:(blk + 1) * WG],
                                    start=(mi == 0), stop=(mi == len(mms) - 1))
                        if use_bc:
                            nc.vector.tensor_tensor(out=psc[:SUB, :], in0=psc[:SUB, :],
                                                    in1=bc_rep[b][:SUB, :], op=ALU.add)
                        st = lnp.tile([SUB, 6], F32, tag="st")
                        nc.vector.bn_stats(out=st[:], in_=psc[:SUB, :])
                        mv = lnp.tile([SUB, 2], F32, tag="mv")
                        nc.vector.bn_aggr(out=mv[:], in_=st[:])
                        sd = lnp.tile([SUB, 1], F32, tag="sd")
                        nc.scalar.activation(out=sd[:], in_=mv[:, 1:2], func=ACTF.Sqrt,
                                             bias=eps_sb[:SUB], scale=1.0)
                        rstd = lnp.tile([SUB, 1], F32, tag="rstd")
                        nc.vector.reciprocal(out=rstd[:], in_=sd[:])
                        nmr = lnp.tile([SUB, 1], F32, tag="nmr")
                        nc.vector.tensor_scalar(out=nmr[:], in0=mv[:, 0:1],
                                                scalar1=rstd[:], scalar2=-1.0,
                                                op0=ALU.mult, op1=ALU.mult)
                        xhat = lnp.tile([SUB, W], F32, tag="xhat")
                        nc.scalar.activation(out=xhat[:], in_=psc[:SUB, :],
                                             func=ACTF.Identity,
                                             bias=nmr[:], scale=rstd[:])
                        scr = lnp.tile([SUB, W], F32, tag="scr")
                        qt = lnp.tile([SUB, 1], F32, tag="qt")
                        nc.vector.scalar_tensor_tensor(
                            out=scr[:], in0=xhat[:], scalar=wcol[:],
                            in1=gpw_rep[b][:SUB, :], op0=ALU.mult, op1=ALU.mult,
                            accum_out=qt[:])
                        nc.vector.tensor_tensor(out=acc[:SUB, b:b + 1],
                                                in0=acc[:SUB, b:b + 1], in1=qt[:],
                                                op=ALU.add)
                    nc.vector.tensor_tensor(out=acc[:SUB, NBR:NBR + 1],
                                            in0=acc[:SUB, NBR:NBR + 1], in1=wcol[:],
                                            op=ALU.add)

                # ---- finalize sequence: partition-reduce, divide, project ----
                f_ps_t = psZ.tile([NT, SUB], F32, tag="z", name=f"fin{seq}")
                f_ps = f_ps_t[:NBR + 1, :1]
                nc.tensor.matmul(f_ps[:], acc[:], ones32[:], start=True, stop=True)
                f_sb = accp.tile([NBR + 1, 1], F32, tag="fsb")
                nc.vector.tensor_copy(out=f_sb[:], in_=f_ps[:])
                frow = accp.tile([1, NBR + 1], F32, tag="frow")
                nc.sync.dma_start(out=frow[:, :], in_=f_sb[:, :])
                fmax = accp.tile([1, 1], F32, tag="fmax")
                nc.vector.tensor_scalar_max(out=fmax[:], in0=frow[:, NBR:NBR + 1],
                                            scalar1=1.0)
                rec = accp.tile([1, 1], F32, tag="rec")
                nc.vector.reciprocal(out=rec[:], in_=fmax[:])
                feats = accp.tile([1, NBR], F32, tag="feats")
                nc.vector.tensor_scalar(out=feats[:], in0=frow[:, :NBR],
                                        scalar1=rec[:], scalar2=1.0,
                                        op0=ALU.mult, op1=ALU.mult)
                nc.vector.tensor_tensor(out=feats[:], in0=feats[:], in1=fold_sb[:],
                                        op=ALU.add)
                nc.sync.dma_start(out=y[seq:seq + 1, :], in_=feats[:])
    nc.compile()
    return nc


_CACHE = {}


def _get_nc(use_bc: bool):
    if use_bc not in _CACHE:
        _CACHE[use_bc] = _build(use_bc)
    return _CACHE[use_bc]


def kernel(tokens, mask, emb, branch_params):
    tokens = np.asarray(tokens)
    mask = np.asarray(mask)
    emb = np.asarray(emb, dtype=np.float32)
    bps = [{k: np.asarray(v, dtype=np.float32) for k, v in bp.items()}
           for bp in branch_params]

    # ---- host-side packing (shared across cores) ----
    emb16 = emb.astype(np.float16)
    wo_all = np.concatenate([bp['wo'] for bp in bps], axis=1)        # [512, 24]
    woP = np.concatenate([wo_all[cc * 128:(cc + 1) * 128, :] for cc in range(CC)],
                         axis=1).astype(np.float16)                  # [128, 96]
    boP = np.concatenate([bp['bo'] for bp in bps])[:, None].astype(np.float32)
    wcP = np.zeros((128, NT * G * 2 * WG), np.float16)
    for bi, bp in enumerate(bps):
        K = KS[bi]
        for kj in range(K):
            kk = BR_OFF[bi] + kj
            for g in range(G):
                for ic in range(2):
                    blk = (kk * G + g) * 2 + ic
                    # wc[g, o, i, k] -> lhsT [i(128), o(256)]
                    wcP[:, blk * WG:(blk + 1) * WG] = \
                        bp['wc'][g, :, ic * 128:(ic + 1) * 128, kj].T
    cmatP = np.zeros((128, FREE), np.float32)
    tl = np.arange(128, dtype=np.float32)[:, None]
    sl_ = np.arange(SUB, dtype=np.float32)[None, :]
    for kk in range(NT):
        cmatP[:, kk * SUB:(kk + 1) * SUB] = tl - HALO - sl_ - TAPS[kk]
    gpwP = np.stack([bp['ln_g'] * bp['pw'] for bp in bps]).astype(np.float32)
    bcP = np.stack([bp['bc'] for bp in bps]).astype(np.float32)
    foldP = np.array([[float((bp['ln_b'] * bp['pw']).sum() + bp['pb'])
                       for bp in bps]], np.float32)
    use_bc = bool(np.any(bcP))

    in_maps = []
    for c in range(N_CORES):
        tokp = np.full((BPC, ROWS), SENT, np.int32)
        tokp[:, HALO:HALO + S] = tokens[c * BPC:(c + 1) * BPC].astype(np.int32)
        mf = np.zeros((BPC, SPAD), np.float32)
        mf[:, :S] = 1.0 - mask[c * BPC:(c + 1) * BPC].astype(np.float32)
        in_maps.append(dict(
            tokp=tokp.reshape(-1, 1), maskf=mf.reshape(-1, 1), emb16=emb16,
            woP=woP, boP=boP, wcP=wcP, cmatP=cmatP, gpwP=gpwP, bcP=bcP,
            foldP=foldP))

    nc = _get_nc(use_bc)
    res = bass_utils.run_bass_kernel_spmd(nc, in_maps, core_ids=list(range(N_CORES)))
    out = np.concatenate([res.results[c]["y"] for c in range(N_CORES)], axis=0)
    return out.astype(np.float32)
